# revision 37
# baseline (speedup 1.0000x reference)
"""Trainium2 Bass kernel for nn_CPAMDec_Mix (dual cross-attention decoder block).

Math per batch sample b (C=512, C4=128, K=64, N=W*H=4096):
    pv1 = wv @ y1^T + bv          [C, K]   (host-precomputed, scale folded)
    pv2 = wv @ y2^T + bv          [C, K]   (host-precomputed, scale1 folded)
    q^T = wq @ x2 + bq            [C4, N]
    kk  = y2 @ wk^T + bk          [K, C4]  (host-precomputed)
    energy = q @ kk^T             [N, K]
    att = softmax(|energy|, -1)   [N, K]
    out1 = scale  * pv1 @ att^T + x1
    out2 = scale1 * pv2 @ att^T + x2

Sharding: pure data parallel — sample b on core b (B == n_cores == 8).

Final design (evolved v1 87.6 -> v5 66.5 -> v12 ~62-65us via perfetto):
 - Ring-bound: ~15.7 MiB of HBM traffic ~= 46us at 358 GB/s, plus ~8.5us
   of fixed NEFF preamble.  Everything else must hide under the ring.
 - The PE HAM throttle (1.2 vs 2.4 GHz) re-triggers on the micro-stalls
   of any realistic schedule, so the design budget uses COLD matmul rates
   (512-col mm ~= 427 ns).
 - ALL attention chains are hoisted into the load phase (woven so every
   per-engine queue keeps flowing), leaving a pure streaming out-phase.
 - out1/out2 matmuls are ROW-TILED to run concurrently in the PE array:
   pv1 lives in SBUF partitions 0-63 (array rows 0-63), pv2 in 64-127,
   and att^T is materialized twice (partitions 0-63 and 64-127) by
   col-tiled PE transposes, which also run concurrently.  bass infers
   tile_position from the operand base partitions.
 - Out-phase engine budget per quarter vs 5.9us of store drain:
   PE ~5.1 (8 row-tiled po pairs + 4 identity-residual mms), DVE ~5.7
   (o1 epilogues + attention leftovers), ACT ~4.4 (o2 evacs), GPSIMD
   ~4.6 (o2 residual adds for the non-identity chunks).
 - Flat [128, 4096] quarter loads, one [128, 1154] blob for every
   weight/constant, flat half-quarter stores (host packs/unpacks).
"""

import numpy as np
import ml_dtypes

import concourse.bass as bass
import concourse.mybir as mybir
import concourse.tile as tile
from concourse import bacc
from concourse.bass_utils import run_bass_kernel_spmd

F32 = mybir.dt.float32
BF16 = mybir.dt.bfloat16
U32 = mybir.dt.uint32
NP_BF16 = np.dtype(ml_dtypes.bfloat16)
AX = mybir.AxisListType
OP = mybir.AluOpType
AF = mybir.ActivationFunctionType

B, C, W, H, K = 8, 512, 64, 64, 64
C4 = C // 4
N = W * H            # 4096
NT = 512             # columns per f32 psum bank / matmul
NQ = 1024            # quarter width
CC = C // 128        # 4 chunks of 128 over the channel dim
NHALF = NQ // NT     # 2
NQuarters = N // NQ

# blob column offsets (bf16 columns; bq is f32 bitcast at offset 0)
OF_BQ = 0              # [128, 1] f32  == 2 bf16 cols
OF_ID = 2              # [128, 128] identity
OF_WQ = OF_ID + 128    # [128, CC*C4]
OF_KK = OF_WQ + CC * C4   # [128, K] kk^T
OF_PV = OF_KK + K      # [0:64] pv1T, [64:128] pv2T — both [64, C]
BLOB_W = OF_PV + C     # 1154

_CACHE = {}


class _AttQuarter:
    """Attention for one quarter, split into 4 emission stages so the
    chains can be woven into the out-phase emission order."""

    def __init__(self, nc, x2q, ctx):
        self.nc = nc
        self.x2q = x2q
        self.ctx = ctx

    def stage0(self):  # q-projection (NT-granular psum) + qT act
        nc, c = self.nc, self.ctx
        for half in range(NHALF):
            ph = c["psq"].tile([C4, NT], F32, tag="psq")
            o = half * NT
            for cc in range(CC):
                nc.tensor.matmul(
                    ph[:],
                    lhsT=c["wqT"][:, cc * C4 : (cc + 1) * C4],
                    rhs=self.x2q[:, cc * NQ + o : cc * NQ + o + NT],
                    start=(cc == 0),
                    stop=(cc == CC - 1),
                )
            if half == 0:
                self.qT = c["qpool"].tile([C4, NQ], BF16, tag="qT")
            nc.scalar.activation(
                self.qT[:, o : o + NT], ph[:], AF.Identity, bias=c["bq"]
            )

    def stage1(self):  # energy + |e| (ACT) + exp (ACT)
        nc, c = self.nc, self.ctx
        self.psum_e = c["ept"].tile([128, 8 * K], F32, tag="ept")
        for s in range(8):
            nc.tensor.matmul(
                self.psum_e[:, s * K : (s + 1) * K],
                lhsT=self.qT[:, s * 128 : (s + 1) * 128],
                rhs=c["kkT"],
                start=True,
                stop=True,
            )
        self.eabs = c["spool"].tile([128, 8 * K], F32, tag="eabs")
        nc.scalar.activation(self.eabs[:], self.psum_e[:], AF.Abs)
        self.eexp = c["spool"].tile([128, 8 * K], BF16, tag="eexp")
        nc.scalar.activation(self.eexp[:], self.eabs[:], AF.Exp)

    def stage2(self):  # softmax normalize + col-tiled double transpose
        nc, c = self.nc, self.ctx
        rsum = c["spool"].tile([128, 8], F32, tag="rsum")
        nc.vector.tensor_reduce(
            rsum[:],
            self.eexp[:].rearrange("p (g d) -> p g d", g=8),
            axis=AX.X,
            op=OP.add,
        )
        rrec = c["spool"].tile([128, 8], F32, tag="rrec")
        nc.vector.reciprocal(rrec[:], rsum[:])
        att = c["spool"].tile([128, 8 * K], BF16, tag="att")
        # normalize on GPSIMD (SBUF-only op): DVE is the out-phase cadence
        # cap at ~7.05us/quarter while GPSIMD has ~2.5us/q of slack
        nc.gpsimd.tensor_tensor(
            att[:].rearrange("p (g d) -> p g d", g=8),
            self.eexp[:].rearrange("p (g d) -> p g d", g=8),
            rrec[:].unsqueeze(2).broadcast_to((128, 8, K)),
            op=OP.mult,
        )
        # att^T twice: partitions 0-63 for the row-tiled out1 matmuls,
        # 64-127 for out2.  The two copies land in different PE column
        # groups, so the transposes run concurrently (col tiling).
        self.psum_t = c["ept"].tile([128, NQ], BF16, tag="ept")
        for s in range(8):
            nc.tensor.transpose(
                self.psum_t[0:K, s * 128 : (s + 1) * 128],
                att[:, s * K : (s + 1) * K],
                c["ident"],
            )
        for s in range(8):
            nc.tensor.transpose(
                self.psum_t[K : 2 * K, s * 128 : (s + 1) * 128],
                att[:, s * K : (s + 1) * K],
                c["ident"],
            )

    def stage3(self):  # attT (both copies) -> SBUF
        nc, c = self.nc, self.ctx
        self.aT = c["apool"].tile([128, NQ], BF16, tag="attT")
        nc.vector.tensor_copy(
            self.aT[:].bitcast(U32), self.psum_t[:].bitcast(U32)
        )
        return self.aT


def _build_nc():
    nc = bacc.Bacc("TRN2", target_bir_lowering=False, debug=False)

    # x1/x2 arrive host-rearranged to the SBUF tile layout:
    # xr[p, q*4096 + cc*1024 + n] = x[cc*128 + p, q*1024 + n]
    # so each quarter's load is one flat [128, 4096] slice.  out1/out2 use
    # the SAME flat layout (host un-rearranges after the run).
    x1_d = nc.dram_tensor("x1", [128, N * CC], BF16, kind="ExternalInput")
    x2_d = nc.dram_tensor("x2", [128, N * CC], BF16, kind="ExternalInput")
    blob_d = nc.dram_tensor("blob", [128, BLOB_W], BF16, kind="ExternalInput")
    out1_d = nc.dram_tensor("out1", [128, N * CC], BF16, kind="ExternalOutput")
    out2_d = nc.dram_tensor("out2", [128, N * CC], BF16, kind="ExternalOutput")

    with tile.TileContext(nc) as tc:
        with (
            tc.tile_pool(name="const", bufs=1) as const,
            tc.tile_pool(name="qpool", bufs=3) as qpool,
            tc.tile_pool(name="spool", bufs=4) as spool,
            tc.tile_pool(name="apool", bufs=4) as apool,
            tc.tile_pool(name="o1pool", bufs=4) as o1pool,
            tc.tile_pool(name="o2pool", bufs=4) as o2pool,
            tc.tile_pool(name="epool", bufs=6) as epool,
            tc.tile_pool(name="psq", bufs=1, space="PSUM") as psq,
            tc.tile_pool(name="ept", bufs=2, space="PSUM") as ept,
            tc.tile_pool(name="pso", bufs=5, space="PSUM") as pso,
        ):
            # ---- blob in two slices: the qproj part (bq/ident/wq) lands
            # before x2_0 so attention-0 starts as early as possible ----
            blob = const.tile([128, BLOB_W], BF16)
            nc.sync.dma_start(
                out=blob[:, 0:OF_KK], in_=blob_d[:, 0:OF_KK]
            )

            bq_sb = blob[:, OF_BQ : OF_BQ + 2].bitcast(F32)
            ident = blob[:, OF_ID : OF_ID + 128]
            wqT_sb = blob[:, OF_WQ : OF_WQ + CC * C4]
            kkT_sb = blob[:, OF_KK : OF_KK + K]

            # ---- all quarter loads queued upfront in deadline order ----
            x1_sb = [None] * NQuarters
            x2_sb = [None] * NQuarters

            def _load_quarter(dram, q, tag):
                t = const.tile([128, CC * NQ], BF16, tag=tag)
                nc.sync.dma_start(
                    out=t[:], in_=dram[:, q * CC * NQ : (q + 1) * CC * NQ]
                )
                return t

            x2_sb[0] = _load_quarter(x2_d, 0, "x2_0")
            nc.sync.dma_start(
                out=blob[:, OF_KK:BLOB_W], in_=blob_d[:, OF_KK:BLOB_W]
            )
            x2_sb[1] = _load_quarter(x2_d, 1, "x2_1")
            x1_sb[0] = _load_quarter(x1_d, 0, "x1_0")
            x2_sb[2] = _load_quarter(x2_d, 2, "x2_2")
            x1_sb[1] = _load_quarter(x1_d, 1, "x1_1")
            x2_sb[3] = _load_quarter(x2_d, 3, "x2_3")
            x1_sb[2] = _load_quarter(x1_d, 2, "x1_2")
            x1_sb[3] = _load_quarter(x1_d, 3, "x1_3")

            ctx = {
                "psq": psq, "ept": ept, "qpool": qpool, "spool": spool,
                "apool": apool, "wqT": wqT_sb, "kkT": kkT_sb,
                "bq": bq_sb, "ident": ident,
            }
            atts = [_AttQuarter(nc, x2_sb[j], ctx) for j in range(NQuarters)]
            aTs = [None] * NQuarters

            def out_chunk(q, cc, o1, o2):
                aT = aTs[q]
                pv1c = blob[0:K, OF_PV + cc * 128 : OF_PV + (cc + 1) * 128]
                pv2c = blob[K : 2 * K, OF_PV + cc * 128 : OF_PV + (cc + 1) * 128]
                # PE-identity chunks must come FIRST within the quarter:
                # putting the slow GPSIMD adds first serializes the later
                # ACT writes into the same o2 tile behind them (measured
                # 66.5 -> 86.3us).  The last quarter skips GPSIMD entirely
                # so the final store isn't gated by the slow add.
                via_pe = cc < 2 or q == NQuarters - 1
                for i in range(NHALF):
                    ns = slice(cc * NQ + i * NT, cc * NQ + (i + 1) * NT)
                    nt = slice(i * NT, (i + 1) * NT)
                    # row-tiled pair: po1 in PE rows 0-63, po2 in 64-127,
                    # executing concurrently
                    po1 = pso.tile([128, NT], F32, tag="po")
                    nc.tensor.matmul(
                        po1[:], lhsT=pv1c, rhs=aT[0:K, nt],
                        start=True, stop=True,
                    )
                    po2 = pso.tile([128, NT], F32, tag="po")
                    nc.tensor.matmul(
                        po2[:], lhsT=pv2c, rhs=aT[K : 2 * K, nt],
                        start=True, stop=not via_pe,
                    )
                    # o1 residual fused into the DVE evacuation
                    nc.vector.tensor_tensor(
                        o1[:, ns], po1[:], x1_sb[q][:, ns], op=OP.add,
                    )
                    # o2 residual: PE identity-accum for half the chunks,
                    # idle GPSIMD for the rest.  ACT always evacuates.
                    if via_pe:
                        nc.tensor.matmul(
                            po2[:], lhsT=ident, rhs=x2_sb[q][:, ns],
                            start=False, stop=True,
                        )
                        nc.scalar.activation(o2[:, ns], po2[:], AF.Identity)
                    else:
                        o2e = epool.tile([128, NT], BF16, tag="o2e")
                        nc.scalar.activation(o2e[:], po2[:], AF.Identity)
                        nc.gpsimd.tensor_tensor(
                            o2[:, ns], o2e[:], x2_sb[q][:, ns], op=OP.add,
                        )

            def store_half(q, h, o1, o2):
                lo = q * CC * NQ + h * 2 * NQ
                nc.sync.dma_start(
                    out=out1_d[:, lo : lo + 2 * NQ],
                    in_=o1[:, h * 2 * NQ : (h + 1) * 2 * NQ],
                )
                nc.sync.dma_start(
                    out=out2_d[:, lo : lo + 2 * NQ],
                    in_=o2[:, h * 2 * NQ : (h + 1) * 2 * NQ],
                )

            # ---- attention front-run (keeps every engine queue flowing;
            # chain 0 is the critical path so its stages go first — but
            # qp1 fills the PE while chain-0's softmax runs on ACT/DVE) ----
            atts[0].stage0()
            atts[0].stage1()
            atts[1].stage0()
            atts[0].stage2()
            aTs[0] = atts[0].stage3()

            oo = {}
            for q in (0, 1, 2, 3):
                o1t = o1pool.tile([128, CC * NQ], BF16, tag="o1")
                o2t = o2pool.tile([128, CC * NQ], BF16, tag="o2")
                oo[q] = (o1t, o2t)
                # quarters 2/3 run pure; 0/1 are woven with attention tails
                if q == 0:
                    out_chunk(0, 0, *oo[0]); atts[1].stage1()
                    out_chunk(0, 1, *oo[0]); store_half(0, 0, *oo[0])
                    atts[1].stage2(); aTs[1] = atts[1].stage3()
                    out_chunk(0, 2, *oo[0]); atts[2].stage0()
                    out_chunk(0, 3, *oo[0]); store_half(0, 1, *oo[0])
                    atts[2].stage1()
                elif q == 1:
                    out_chunk(1, 0, *oo[1])
                    atts[2].stage2(); aTs[2] = atts[2].stage3()
                    out_chunk(1, 1, *oo[1]); store_half(1, 0, *oo[1])
                    atts[3].stage0(); atts[3].stage1()
                    out_chunk(1, 2, *oo[1])
                    atts[3].stage2(); aTs[3] = atts[3].stage3()
                    out_chunk(1, 3, *oo[1]); store_half(1, 1, *oo[1])
                elif q == 2:
                    out_chunk(q, 0, *oo[q])
                    out_chunk(q, 1, *oo[q]); store_half(q, 0, *oo[q])
                    out_chunk(q, 2, *oo[q])
                    out_chunk(q, 3, *oo[q]); store_half(q, 1, *oo[q])
                else:
                    # last quarter: store at chunk granularity so the very
                    # last store is only 256 KiB deep after compute ends
                    out_chunk(q, 0, *oo[q])
                    out_chunk(q, 1, *oo[q]); store_half(q, 0, *oo[q])
                    out_chunk(q, 2, *oo[q])
                    for t_d, o_t in ((out1_d, oo[q][0]), (out2_d, oo[q][1])):
                        nc.sync.dma_start(
                            out=t_d[:, q * CC * NQ + 2 * NQ : q * CC * NQ + 3 * NQ],
                            in_=o_t[:, 2 * NQ : 3 * NQ],
                        )
                    out_chunk(q, 3, *oo[q])
                    for t_d, o_t in ((out1_d, oo[q][0]), (out2_d, oo[q][1])):
                        nc.sync.dma_start(
                            out=t_d[:, q * CC * NQ + 3 * NQ : (q + 1) * CC * NQ],
                            in_=o_t[:, 3 * NQ : CC * NQ],
                        )
    nc.compile()
    return nc


def _get_nc():
    if "nc" not in _CACHE:
        _CACHE["nc"] = _build_nc()
    return _CACHE["nc"]


def _rearr(x):
    # [C, N] -> [128, q*4096 + cc*1024 + n] (SBUF quarter-tile layout)
    return np.ascontiguousarray(
        x.reshape(CC, 128, N // NQ, NQ).transpose(1, 2, 0, 3).reshape(128, N * CC)
    )


def _unrearr(t):
    # inverse of _rearr: [128, N*CC] -> [C, N]
    return t.reshape(128, N // NQ, CC, NQ).transpose(2, 0, 1, 3).reshape(C, N)


def kernel(x1, y1, x2, y2, wq, bq, wk, bk, wv, bv, scale, scale1, **run_kwargs):
    x1 = np.asarray(x1, np.float32).astype(NP_BF16)
    x2 = np.asarray(x2, np.float32).astype(NP_BF16)
    y1 = np.asarray(y1, np.float32)
    y2 = np.asarray(y2, np.float32)
    wq = np.asarray(wq, np.float32)
    wk = np.asarray(wk, np.float32)
    wv = np.asarray(wv, np.float32)
    bv_ = np.asarray(bv, np.float32).reshape(C)
    bk_ = np.asarray(bk, np.float32).reshape(C4)
    sc1 = float(np.asarray(scale).reshape(-1)[0])
    sc2 = float(np.asarray(scale1).reshape(-1)[0])

    def _chunked(m, inner):
        # [CC*128, inner] -> [128, CC*inner]: dst[p, cc*inner+j] = m[cc*128+p, j]
        return m.reshape(CC, 128, inner).transpose(1, 0, 2).reshape(128, CC * inner)

    blob_shared = np.zeros((128, BLOB_W), NP_BF16)
    blob_shared[:, OF_BQ : OF_BQ + 2] = (
        np.ascontiguousarray(np.asarray(bq, np.float32).reshape(C4, 1))
        .view(np.uint16)
        .view(NP_BF16)
    )
    blob_shared[:, OF_ID : OF_ID + 128] = np.eye(128, dtype=np.float32).astype(NP_BF16)
    blob_shared[:, OF_WQ : OF_WQ + CC * C4] = _chunked(wq.T, C4).astype(NP_BF16)

    in_maps = []
    for b in range(B):
        blob = blob_shared.copy()
        # kk^T[d, k] = wk @ y2[b]^T + bk  (f32 on host, stored bf16)
        blob[:, OF_KK : OF_KK + K] = (wk @ y2[b].T + bk_[:, None]).astype(NP_BF16)
        # pv^T[k, c] = scale * (y @ wv^T + bv); pv1 in partitions 0-63,
        # pv2 in 64-127 (PE row-tiling)
        blob[0:K, OF_PV : OF_PV + C] = (sc1 * (y1[b] @ wv.T + bv_)).astype(NP_BF16)
        blob[K : 2 * K, OF_PV : OF_PV + C] = (
            sc2 * (y2[b] @ wv.T + bv_)
        ).astype(NP_BF16)
        in_maps.append(
            {
                "x1": _rearr(x1[b].reshape(C, N)),
                "x2": _rearr(x2[b].reshape(C, N)),
                "blob": blob,
            }
        )
    nc = _get_nc()
    res = run_bass_kernel_spmd(nc, in_maps, list(range(B)), **run_kwargs)
    _CACHE["last_results"] = res
    out1 = np.stack(
        [
            _unrearr(res.results[b]["out1"].astype(np.float32)).reshape(C, W, H)
            for b in range(B)
        ]
    )
    out2 = np.stack(
        [
            _unrearr(res.results[b]["out2"].astype(np.float32)).reshape(C, W, H)
            for b in range(B)
        ]
    )
    return (out1, out2)


# revision 38
# speedup vs baseline: 1.0871x; 1.0871x over previous
"""Trainium2 Bass kernel for nn_CPAMDec_Mix (dual cross-attention decoder block).

Math per batch sample b (C=512, C4=128, K=64, N=W*H=4096):
    pv1 = wv @ y1^T + bv          [C, K]   (host-precomputed, scale folded)
    pv2 = wv @ y2^T + bv          [C, K]   (host-precomputed, scale1 folded)
    q^T = wq @ x2 + bq            [C4, N]
    kk  = y2 @ wk^T + bk          [K, C4]  (host-precomputed)
    energy = q @ kk^T             [N, K]
    att = softmax(|energy|, -1)   [N, K]
    out1 = scale  * pv1 @ att^T + x1
    out2 = scale1 * pv2 @ att^T + x2

Sharding: pure data parallel — sample b on core b (B == n_cores == 8).

Final design (evolved v1 87.6 -> v5 66.5 -> v12 ~62-65us via perfetto):
 - Ring-bound: ~15.7 MiB of HBM traffic ~= 46us at 358 GB/s, plus ~8.5us
   of fixed NEFF preamble.  Everything else must hide under the ring.
 - The PE HAM throttle (1.2 vs 2.4 GHz) re-triggers on the micro-stalls
   of any realistic schedule, so the design budget uses COLD matmul rates
   (512-col mm ~= 427 ns).
 - ALL attention chains are hoisted into the load phase (woven so every
   per-engine queue keeps flowing), leaving a pure streaming out-phase.
 - out1/out2 matmuls are ROW-TILED to run concurrently in the PE array:
   pv1 lives in SBUF partitions 0-63 (array rows 0-63), pv2 in 64-127,
   and att^T is materialized twice (partitions 0-63 and 64-127) by
   col-tiled PE transposes, which also run concurrently.  bass infers
   tile_position from the operand base partitions.
 - Out-phase engine budget per quarter vs 5.9us of store drain:
   PE ~5.1 (8 row-tiled po pairs + 4 identity-residual mms), DVE ~5.7
   (o1 epilogues + attention leftovers), ACT ~4.4 (o2 evacs), GPSIMD
   ~4.6 (o2 residual adds for the non-identity chunks).
 - Flat [128, 4096] quarter loads, one [128, 1154] blob for every
   weight/constant, flat half-quarter stores (host packs/unpacks).
"""

import numpy as np
import ml_dtypes

import concourse.bass as bass
import concourse.mybir as mybir
import concourse.tile as tile
from concourse import bacc
from concourse.bass_utils import run_bass_kernel_spmd

F32 = mybir.dt.float32
BF16 = mybir.dt.bfloat16
U32 = mybir.dt.uint32
NP_BF16 = np.dtype(ml_dtypes.bfloat16)
AX = mybir.AxisListType
OP = mybir.AluOpType
AF = mybir.ActivationFunctionType

B, C, W, H, K = 8, 512, 64, 64, 64
C4 = C // 4
N = W * H            # 4096
NT = 512             # columns per f32 psum bank / matmul
NQ = 1024            # quarter width
CC = C // 128        # 4 chunks of 128 over the channel dim
NHALF = NQ // NT     # 2
NQuarters = N // NQ

# blob column offsets (bf16 columns; bq is f32 bitcast at offset 0)
OF_BQ = 0              # [128, 1] f32  == 2 bf16 cols
OF_ID = 2              # [128, 128] identity
OF_WQ = OF_ID + 128    # [128, CC*C4]
OF_KK = OF_WQ + CC * C4   # [128, K] kk^T
OF_PV = OF_KK + K      # [0:64] pv1T, [64:128] pv2T — both [64, C]
BLOB_W = OF_PV + C     # 1154

_CACHE = {}


class _AttQuarter:
    """Attention for one quarter, split into 4 emission stages so the
    chains can be woven into the out-phase emission order."""

    def __init__(self, nc, x2q, ctx):
        self.nc = nc
        self.x2q = x2q
        self.ctx = ctx

    def stage0(self):  # q-projection (NT-granular psum) + qT act
        nc, c = self.nc, self.ctx
        for half in range(NHALF):
            ph = c["psq"].tile([C4, NT], F32, tag="psq")
            o = half * NT
            for cc in range(CC):
                nc.tensor.matmul(
                    ph[:],
                    lhsT=c["wqT"][:, cc * C4 : (cc + 1) * C4],
                    rhs=self.x2q[:, cc * NQ + o : cc * NQ + o + NT],
                    start=(cc == 0),
                    stop=(cc == CC - 1),
                )
            if half == 0:
                self.qT = c["qpool"].tile([C4, NQ], BF16, tag="qT")
            nc.scalar.activation(
                self.qT[:, o : o + NT], ph[:], AF.Identity, bias=c["bq"]
            )

    def stage1(self):  # energy + |e| (ACT) + exp (ACT)
        nc, c = self.nc, self.ctx
        self.psum_e = c["ept"].tile([128, 8 * K], F32, tag="ept")
        for s in range(8):
            nc.tensor.matmul(
                self.psum_e[:, s * K : (s + 1) * K],
                lhsT=self.qT[:, s * 128 : (s + 1) * 128],
                rhs=c["kkT"],
                start=True,
                stop=True,
            )
        self.eabs = c["spool"].tile([128, 8 * K], F32, tag="eabs")
        nc.scalar.activation(self.eabs[:], self.psum_e[:], AF.Abs)
        self.eexp = c["spool"].tile([128, 8 * K], BF16, tag="eexp")
        nc.scalar.activation(self.eexp[:], self.eabs[:], AF.Exp)

    def stage2(self):  # softmax normalize + col-tiled double transpose
        nc, c = self.nc, self.ctx
        rsum = c["spool"].tile([128, 8], F32, tag="rsum")
        nc.vector.tensor_reduce(
            rsum[:],
            self.eexp[:].rearrange("p (g d) -> p g d", g=8),
            axis=AX.X,
            op=OP.add,
        )
        rrec = c["spool"].tile([128, 8], F32, tag="rrec")
        nc.vector.reciprocal(rrec[:], rsum[:])
        att = c["spool"].tile([128, 8 * K], BF16, tag="att")
        nc.vector.tensor_tensor(
            att[:].rearrange("p (g d) -> p g d", g=8),
            self.eexp[:].rearrange("p (g d) -> p g d", g=8),
            rrec[:].unsqueeze(2).broadcast_to((128, 8, K)),
            op=OP.mult,
        )
        # att^T twice: partitions 0-63 for the row-tiled out1 matmuls,
        # 64-127 for out2.  The two copies land in different PE column
        # groups, so the transposes run concurrently (col tiling).
        self.psum_t = c["ept"].tile([128, NQ], BF16, tag="ept")
        for s in range(8):
            nc.tensor.transpose(
                self.psum_t[0:K, s * 128 : (s + 1) * 128],
                att[:, s * K : (s + 1) * K],
                c["ident"],
            )
        for s in range(8):
            nc.tensor.transpose(
                self.psum_t[K : 2 * K, s * 128 : (s + 1) * 128],
                att[:, s * K : (s + 1) * K],
                c["ident"],
            )

    def stage3(self):  # attT (both copies) -> SBUF
        nc, c = self.nc, self.ctx
        self.aT = c["apool"].tile([128, NQ], BF16, tag="attT")
        nc.vector.tensor_copy(
            self.aT[:].bitcast(U32), self.psum_t[:].bitcast(U32)
        )
        return self.aT


def _build_nc():
    nc = bacc.Bacc("TRN2", target_bir_lowering=False, debug=False)

    # x1/x2 arrive host-rearranged to the SBUF tile layout:
    # xr[p, q*4096 + cc*1024 + n] = x[cc*128 + p, q*1024 + n]
    # so each quarter's load is one flat [128, 4096] slice.  out1/out2 use
    # the SAME flat layout (host un-rearranges after the run).
    x1_d = nc.dram_tensor("x1", [128, N * CC], BF16, kind="ExternalInput")
    x2_d = nc.dram_tensor("x2", [128, N * CC], BF16, kind="ExternalInput")
    blob_d = nc.dram_tensor("blob", [128, BLOB_W], BF16, kind="ExternalInput")
    out1_d = nc.dram_tensor("out1", [128, N * CC], BF16, kind="ExternalOutput")
    out2_d = nc.dram_tensor("out2", [128, N * CC], BF16, kind="ExternalOutput")

    with tile.TileContext(nc) as tc:
        with (
            tc.tile_pool(name="const", bufs=1) as const,
            tc.tile_pool(name="qpool", bufs=3) as qpool,
            tc.tile_pool(name="spool", bufs=4) as spool,
            tc.tile_pool(name="apool", bufs=4) as apool,
            tc.tile_pool(name="o1pool", bufs=4) as o1pool,
            tc.tile_pool(name="o2pool", bufs=4) as o2pool,
            tc.tile_pool(name="epool", bufs=6) as epool,
            tc.tile_pool(name="psq", bufs=1, space="PSUM") as psq,
            tc.tile_pool(name="ept", bufs=2, space="PSUM") as ept,
            tc.tile_pool(name="pso", bufs=5, space="PSUM") as pso,
        ):
            # ---- blob in two slices: the qproj part (bq/ident/wq) lands
            # before x2_0 so attention-0 starts as early as possible ----
            blob = const.tile([128, BLOB_W], BF16)
            nc.sync.dma_start(
                out=blob[:, 0:OF_KK], in_=blob_d[:, 0:OF_KK]
            )

            bq_sb = blob[:, OF_BQ : OF_BQ + 2].bitcast(F32)
            ident = blob[:, OF_ID : OF_ID + 128]
            wqT_sb = blob[:, OF_WQ : OF_WQ + CC * C4]
            kkT_sb = blob[:, OF_KK : OF_KK + K]

            # ---- all quarter loads queued upfront in deadline order ----
            x1_sb = [None] * NQuarters
            x2_sb = [None] * NQuarters

            def _load_quarter(dram, q, tag):
                t = const.tile([128, CC * NQ], BF16, tag=tag)
                nc.sync.dma_start(
                    out=t[:], in_=dram[:, q * CC * NQ : (q + 1) * CC * NQ]
                )
                return t

            x2_sb[0] = _load_quarter(x2_d, 0, "x2_0")
            nc.sync.dma_start(
                out=blob[:, OF_KK:BLOB_W], in_=blob_d[:, OF_KK:BLOB_W]
            )
            x2_sb[1] = _load_quarter(x2_d, 1, "x2_1")
            x1_sb[0] = _load_quarter(x1_d, 0, "x1_0")
            x2_sb[2] = _load_quarter(x2_d, 2, "x2_2")
            x1_sb[1] = _load_quarter(x1_d, 1, "x1_1")
            x2_sb[3] = _load_quarter(x2_d, 3, "x2_3")
            x1_sb[2] = _load_quarter(x1_d, 2, "x1_2")
            x1_sb[3] = _load_quarter(x1_d, 3, "x1_3")

            ctx = {
                "psq": psq, "ept": ept, "qpool": qpool, "spool": spool,
                "apool": apool, "wqT": wqT_sb, "kkT": kkT_sb,
                "bq": bq_sb, "ident": ident,
            }
            atts = [_AttQuarter(nc, x2_sb[j], ctx) for j in range(NQuarters)]
            aTs = [None] * NQuarters

            def out_chunk(q, cc, o1, o2):
                aT = aTs[q]
                pv1c = blob[0:K, OF_PV + cc * 128 : OF_PV + (cc + 1) * 128]
                pv2c = blob[K : 2 * K, OF_PV + cc * 128 : OF_PV + (cc + 1) * 128]
                # PE-identity chunks must come FIRST within the quarter:
                # putting the slow GPSIMD adds first serializes the later
                # ACT writes into the same o2 tile behind them (measured
                # 66.5 -> 86.3us).  The last quarter skips GPSIMD entirely
                # so the final store isn't gated by the slow add.
                via_pe = cc < 2 or q == NQuarters - 1
                for i in range(NHALF):
                    ns = slice(cc * NQ + i * NT, cc * NQ + (i + 1) * NT)
                    nt = slice(i * NT, (i + 1) * NT)
                    # row-tiled pair: po1 in PE rows 0-63, po2 in 64-127,
                    # executing concurrently
                    po1 = pso.tile([128, NT], F32, tag="po")
                    nc.tensor.matmul(
                        po1[:], lhsT=pv1c, rhs=aT[0:K, nt],
                        start=True, stop=True,
                    )
                    po2 = pso.tile([128, NT], F32, tag="po")
                    nc.tensor.matmul(
                        po2[:], lhsT=pv2c, rhs=aT[K : 2 * K, nt],
                        start=True, stop=not via_pe,
                    )
                    # o1 residual fused into the DVE evacuation
                    nc.vector.tensor_tensor(
                        o1[:, ns], po1[:], x1_sb[q][:, ns], op=OP.add,
                    )
                    # o2 residual: PE identity-accum for half the chunks,
                    # idle GPSIMD for the rest.  ACT always evacuates.
                    if via_pe:
                        nc.tensor.matmul(
                            po2[:], lhsT=ident, rhs=x2_sb[q][:, ns],
                            start=False, stop=True,
                        )
                        nc.scalar.activation(o2[:, ns], po2[:], AF.Identity)
                    else:
                        o2e = epool.tile([128, NT], BF16, tag="o2e")
                        nc.scalar.activation(o2e[:], po2[:], AF.Identity)
                        nc.gpsimd.tensor_tensor(
                            o2[:, ns], o2e[:], x2_sb[q][:, ns], op=OP.add,
                        )

            def store_half(q, h, o1, o2):
                lo = q * CC * NQ + h * 2 * NQ
                nc.sync.dma_start(
                    out=out1_d[:, lo : lo + 2 * NQ],
                    in_=o1[:, h * 2 * NQ : (h + 1) * 2 * NQ],
                )
                nc.sync.dma_start(
                    out=out2_d[:, lo : lo + 2 * NQ],
                    in_=o2[:, h * 2 * NQ : (h + 1) * 2 * NQ],
                )

            # ---- attention front-run (keeps every engine queue flowing;
            # chain 0 is the critical path so its stages go first — but
            # qp1 fills the PE while chain-0's softmax runs on ACT/DVE) ----
            atts[0].stage0()
            atts[0].stage1()
            atts[1].stage0()
            atts[0].stage2()
            aTs[0] = atts[0].stage3()

            oo = {}
            for q in (0, 1, 2, 3):
                o1t = o1pool.tile([128, CC * NQ], BF16, tag="o1")
                o2t = o2pool.tile([128, CC * NQ], BF16, tag="o2")
                oo[q] = (o1t, o2t)
                # quarters 2/3 run pure; 0/1 are woven with attention tails
                if q == 0:
                    out_chunk(0, 0, *oo[0]); atts[1].stage1()
                    out_chunk(0, 1, *oo[0]); store_half(0, 0, *oo[0])
                    atts[1].stage2(); aTs[1] = atts[1].stage3()
                    out_chunk(0, 2, *oo[0]); atts[2].stage0()
                    out_chunk(0, 3, *oo[0]); store_half(0, 1, *oo[0])
                    atts[2].stage1()
                elif q == 1:
                    out_chunk(1, 0, *oo[1])
                    atts[2].stage2(); aTs[2] = atts[2].stage3()
                    out_chunk(1, 1, *oo[1]); store_half(1, 0, *oo[1])
                    atts[3].stage0(); atts[3].stage1()
                    out_chunk(1, 2, *oo[1])
                    atts[3].stage2(); aTs[3] = atts[3].stage3()
                    out_chunk(1, 3, *oo[1]); store_half(1, 1, *oo[1])
                elif q == 2:
                    out_chunk(q, 0, *oo[q])
                    out_chunk(q, 1, *oo[q]); store_half(q, 0, *oo[q])
                    out_chunk(q, 2, *oo[q])
                    out_chunk(q, 3, *oo[q]); store_half(q, 1, *oo[q])
                else:
                    # last quarter: store at chunk granularity so the very
                    # last store is only 256 KiB deep after compute ends
                    out_chunk(q, 0, *oo[q])
                    out_chunk(q, 1, *oo[q]); store_half(q, 0, *oo[q])
                    out_chunk(q, 2, *oo[q])
                    for t_d, o_t in ((out1_d, oo[q][0]), (out2_d, oo[q][1])):
                        nc.sync.dma_start(
                            out=t_d[:, q * CC * NQ + 2 * NQ : q * CC * NQ + 3 * NQ],
                            in_=o_t[:, 2 * NQ : 3 * NQ],
                        )
                    out_chunk(q, 3, *oo[q])
                    for t_d, o_t in ((out1_d, oo[q][0]), (out2_d, oo[q][1])):
                        nc.sync.dma_start(
                            out=t_d[:, q * CC * NQ + 3 * NQ : (q + 1) * CC * NQ],
                            in_=o_t[:, 3 * NQ : CC * NQ],
                        )
    nc.compile()
    return nc


def _get_nc():
    if "nc" not in _CACHE:
        _CACHE["nc"] = _build_nc()
    return _CACHE["nc"]


def _rearr(x):
    # [C, N] -> [128, q*4096 + cc*1024 + n] (SBUF quarter-tile layout)
    return np.ascontiguousarray(
        x.reshape(CC, 128, N // NQ, NQ).transpose(1, 2, 0, 3).reshape(128, N * CC)
    )


def _unrearr(t):
    # inverse of _rearr: [128, N*CC] -> [C, N]
    return t.reshape(128, N // NQ, CC, NQ).transpose(2, 0, 1, 3).reshape(C, N)


def kernel(x1, y1, x2, y2, wq, bq, wk, bk, wv, bv, scale, scale1, **run_kwargs):
    x1 = np.asarray(x1, np.float32).astype(NP_BF16)
    x2 = np.asarray(x2, np.float32).astype(NP_BF16)
    y1 = np.asarray(y1, np.float32)
    y2 = np.asarray(y2, np.float32)
    wq = np.asarray(wq, np.float32)
    wk = np.asarray(wk, np.float32)
    wv = np.asarray(wv, np.float32)
    bv_ = np.asarray(bv, np.float32).reshape(C)
    bk_ = np.asarray(bk, np.float32).reshape(C4)
    sc1 = float(np.asarray(scale).reshape(-1)[0])
    sc2 = float(np.asarray(scale1).reshape(-1)[0])

    def _chunked(m, inner):
        # [CC*128, inner] -> [128, CC*inner]: dst[p, cc*inner+j] = m[cc*128+p, j]
        return m.reshape(CC, 128, inner).transpose(1, 0, 2).reshape(128, CC * inner)

    blob_shared = np.zeros((128, BLOB_W), NP_BF16)
    blob_shared[:, OF_BQ : OF_BQ + 2] = (
        np.ascontiguousarray(np.asarray(bq, np.float32).reshape(C4, 1))
        .view(np.uint16)
        .view(NP_BF16)
    )
    blob_shared[:, OF_ID : OF_ID + 128] = np.eye(128, dtype=np.float32).astype(NP_BF16)
    blob_shared[:, OF_WQ : OF_WQ + CC * C4] = _chunked(wq.T, C4).astype(NP_BF16)

    in_maps = []
    for b in range(B):
        blob = blob_shared.copy()
        # kk^T[d, k] = wk @ y2[b]^T + bk  (f32 on host, stored bf16)
        blob[:, OF_KK : OF_KK + K] = (wk @ y2[b].T + bk_[:, None]).astype(NP_BF16)
        # pv^T[k, c] = scale * (y @ wv^T + bv); pv1 in partitions 0-63,
        # pv2 in 64-127 (PE row-tiling)
        blob[0:K, OF_PV : OF_PV + C] = (sc1 * (y1[b] @ wv.T + bv_)).astype(NP_BF16)
        blob[K : 2 * K, OF_PV : OF_PV + C] = (
            sc2 * (y2[b] @ wv.T + bv_)
        ).astype(NP_BF16)
        in_maps.append(
            {
                "x1": _rearr(x1[b].reshape(C, N)),
                "x2": _rearr(x2[b].reshape(C, N)),
                "blob": blob,
            }
        )
    nc = _get_nc()
    res = run_bass_kernel_spmd(nc, in_maps, list(range(B)), **run_kwargs)
    _CACHE["last_results"] = res
    out1 = np.stack(
        [
            _unrearr(res.results[b]["out1"].astype(np.float32)).reshape(C, W, H)
            for b in range(B)
        ]
    )
    out2 = np.stack(
        [
            _unrearr(res.results[b]["out2"].astype(np.float32)).reshape(C, W, H)
            for b in range(B)
        ]
    )
    return (out1, out2)
